# revision 1
# baseline (speedup 1.0000x reference)
"""Segment-prefix max kernel for Trainium2 (8 NeuronCores, SPMD).

Problem: x [1048576, 128] f32, 2048 uniform segments of 512 rows each;
out[i, :] = max over the first (512 - window_size + 1) rows of segment i.

Strategy (memory-bound, ~512 MiB streamed from HBM):
  - Shard segments across 8 cores: core c gets rows [c*131072, (c+1)*131072)
    and produces out rows [c*256, (c+1)*256). No cross-core communication.
  - Per core, each 512-row segment is loaded so SBUF partition p holds rows
    {4p..4p+3} of the segment (2 KiB contiguous DRAM runs); 2 MiB tiles of
    8 segments, alternating the SP and Activation HWDGE rings.
  - Three DVE tensor_max ops fold the 4 rows per partition down to 1,
    excluding the window's last rows via partition-sliced operands.
  - The cross-partition max (128 -> 1) runs through PE transposes (identity
    matmul), 4 segments into one PSUM bank, then a single DVE reduce_max
    along the free axis yields 4 output columns at once.
  - Columns accumulate in an SBUF [128, n_seg] buffer that is PE-transposed
    back to row-major [n_seg, 128] chunks and DMA'd out.
"""

import numpy as np

import concourse.bacc as bacc
import concourse.bass as bass
import concourse.tile as tile
from concourse import mybir
from concourse.bass_utils import run_bass_kernel_spmd
from concourse.masks import make_identity

N_CORES = 8
SEG_LEN = 512
D = 128
J = 4  # segment rows stacked per partition (J * 128 partitions = 512 rows)
SEGS_PER_TILE = 8  # 8 segments * 512 rows * 128 * 4 B = 2 MiB per DMA load

_PROGRAM_CACHE: dict = {}


def _build_program(n_seg_core: int, count: int) -> bacc.Bacc:
    """Bass program for one core: n_seg_core segments, max over first
    `count` rows of each."""
    rows = n_seg_core * SEG_LEN
    n_tiles = n_seg_core // SEGS_PER_TILE
    f32 = mybir.dt.float32

    nc = bacc.Bacc("TRN2", target_bir_lowering=False, debug=False)
    x_in = nc.dram_tensor("x", [rows, D], f32, kind="ExternalInput")
    out_t = nc.dram_tensor("out", [n_seg_core, D], f32, kind="ExternalOutput")

    # row = ((t*S + s)*128 + p)*J + j
    x_v = x_in.rearrange("(t s p j) d -> t p s j d", s=SEGS_PER_TILE, p=128, j=J)

    # valid partitions for j-view: rows J*p + j < count
    v = [max(0, min(128, (count - j + J - 1) // J)) if count > j else 0 for j in range(J)]
    fast = v[0] == 128 and v[1] == 128  # rows 4p, 4p+1 valid everywhere

    with tile.TileContext(nc) as tc:
        with (
            tc.tile_pool(name="io", bufs=8) as io_pool,
            tc.tile_pool(name="work", bufs=4) as work_pool,
            tc.tile_pool(name="psum", bufs=8, space="PSUM") as psum_pool,
            tc.tile_pool(name="consts", bufs=1) as consts,
        ):
            ident = consts.tile([128, 128], f32)
            make_identity(nc, ident)
            outbuf = consts.tile([128, n_seg_core], f32)

            for t in range(n_tiles):
                tl = io_pool.tile([128, SEGS_PER_TILE, J, D], f32, tag="tl")
                hw = nc.sync if t % 2 == 0 else nc.scalar
                hw.dma_start(out=tl, in_=x_v[t])

                acc = work_pool.tile([128, SEGS_PER_TILE, D], f32, tag="acc")
                if fast:
                    nc.vector.tensor_max(
                        out=acc, in0=tl[:, :, 0, :], in1=tl[:, :, 1, :]
                    )
                    for j in range(2, J):
                        if v[j] > 0:
                            nc.vector.tensor_max(
                                out=acc[: v[j]],
                                in0=acc[: v[j]],
                                in1=tl[: v[j], :, j, :],
                            )
                else:
                    nc.vector.memset(acc, float("-inf"))
                    for j in range(J):
                        if v[j] > 0:
                            nc.vector.tensor_max(
                                out=acc[: v[j]],
                                in0=acc[: v[j]],
                                in1=tl[: v[j], :, j, :],
                            )

                for g in range(SEGS_PER_TILE // 4):
                    bank = psum_pool.tile([128, 4, 128], f32, tag="pt")
                    for c in range(4):
                        nc.tensor.transpose(
                            bank[:, c, :], acc[:, g * 4 + c, :], ident
                        )
                    seg = t * SEGS_PER_TILE + g * 4
                    nc.vector.reduce_max(
                        out=outbuf[:, seg : seg + 4], in_=bank,
                        axis=mybir.AxisListType.X,
                    )

            # outbuf is [128 d, n_seg_core]; transpose back to [seg, d] chunks
            for c in range(n_seg_core // 128):
                pt = psum_pool.tile([128, 4, 128], f32, tag="pt")
                nc.tensor.transpose(
                    pt[:, 0, :], outbuf[:, c * 128 : (c + 1) * 128], ident
                )
                ot = io_pool.tile([128, 128], f32, tag="ot")
                nc.scalar.copy(ot, pt[:, 0, :])
                nc.sync.dma_start(out=out_t[c * 128 : (c + 1) * 128, :], in_=ot)
    nc.compile()
    return nc


def _numpy_fallback(x: np.ndarray, sizes: np.ndarray, w: int) -> np.ndarray:
    ends = np.cumsum(sizes)
    starts = ends - sizes
    out = np.full((sizes.shape[0], x.shape[1]), -np.inf, dtype=np.float32)
    for i in range(sizes.shape[0]):
        c = int(sizes[i]) - w + 1
        if c > 0:
            out[i] = x[int(starts[i]) : int(starts[i]) + c].max(axis=0)
    return out


def kernel(x, sizes, window_size) -> np.ndarray:
    x = np.ascontiguousarray(np.asarray(x, dtype=np.float32))
    sizes = np.asarray(sizes)
    w = int(np.asarray(window_size))
    n_seg = sizes.shape[0]
    count = SEG_LEN - w + 1

    uniform = (
        x.ndim == 2
        and x.shape[1] == D
        and bool((sizes == SEG_LEN).all())
        and x.shape[0] == n_seg * SEG_LEN
        and n_seg % (N_CORES * SEGS_PER_TILE) == 0
        and (n_seg // N_CORES) % 128 == 0
        and 0 < count <= SEG_LEN
    )
    if not uniform:
        return _numpy_fallback(x, sizes, w)

    n_seg_core = n_seg // N_CORES
    key = (n_seg_core, count)
    if key not in _PROGRAM_CACHE:
        _PROGRAM_CACHE[key] = _build_program(n_seg_core, count)
    nc = _PROGRAM_CACHE[key]

    shards = np.split(x, N_CORES, axis=0)
    in_maps = [{"x": s} for s in shards]
    res = run_bass_kernel_spmd(nc, in_maps, core_ids=list(range(N_CORES)))
    return np.concatenate([r["out"] for r in res.results], axis=0)

